# revision 26
# baseline (speedup 1.0000x reference)
"""DeepInsight encoding kernel for 8 Trainium2 NeuronCores.

Data-parallel over batch: each core builds 64 interleaved [H, W*5] output
planes in one resident SBUF buffer and streams them to HBM in 17 chunks
(2,2,4x15 planes). The 21MB/core output write at ~370GB/s is the roofline;
everything else is organized to keep the output DMA queue saturated from
~10us until the end:

  - input loads ride separate DMA rings (ACT HWDGE ring for the critical
    head, gpsimd SWDGE ring for the bulk scr tail) so they never queue
    behind output packets on the SP ring,
  - per-batch prep that the baseline loaded as 1MB of replicated bf16
    (bh broadcast, c3 column-minus-row) is instead built on-chip with
    k=1 / k=32 matmuls into PSUM (ones- and -onehot stationaries), cutting
    HBM reads to ~0.6MB,
  - c1 scatter + c2 row-copy are two matmuls per chunk into one PSUM
    tile -> interleaved runs-of-2 strided copies split DVE/ACT,
  - c3 |x_col - x_row| accumulated in PSUM (A - B) -> one ACT Abs with
    4x-replicated stride-5 write,
  - c4 bars: is_gt(PSUM, iota) into a contiguous stage tile, then a
    strided copy; gap zeros via 3 DVE memsets (mid gaps merged into one
    runs-of-2 op),
  - stamp channel copied by gpsimd.
"""

import numpy as np

B, D, H, W, C = 512, 32, 128, 128, 5
NCORES = 8
BPC = B // NCORES            # 64 batches per core
FP = W * C                   # 640 floats per output row
GROUP_SIZES = [2, 2] + [4] * 15
assert sum(GROUP_SIZES) == BPC

# tF [128, 129] f32:   iota 0 | stamp 1:129
_F32W = 129
# tC [32, 8704] bf16: scatR 0:128 | onehotR 128:256 | x_t 256:320 |
#                     negRow 320:448 | xsT 448:512 | scr 512:8704
_SCATR0, _ONEHOTR0, _XT0, _NEGROW0, _XST0, _SCR0, _CW = (
    0, 128, 256, 320, 448, 512, 8704)
# tO [1, 4224] bf16: xs_flat 0:2048 | bh_flat 2048:4096 | ones 4096:4224
_XS0, _BH0, _ONES0, _OW = 0, 2048, 4096, 4224

_RUNNER = None


def _build_nc():
    import concourse.bacc as bacc
    import concourse.mybir as mybir
    from concourse.tile import TileContext

    f32 = mybir.dt.float32
    bf16 = mybir.dt.bfloat16
    alu = mybir.AluOpType
    act = mybir.ActivationFunctionType

    nc = bacc.Bacc()
    dF = nc.dram_tensor("df", [H, _F32W], f32, kind="ExternalInput")
    dC = nc.dram_tensor("dc", [D, _CW], bf16, kind="ExternalInput")
    dO = nc.dram_tensor("do", [1, _OW], bf16, kind="ExternalInput")
    out_d = nc.dram_tensor("out", [BPC, H, FP], f32, kind="ExternalOutput")

    with TileContext(nc) as tc:
        with (
            tc.tile_pool(name="const", bufs=1) as cpool,
            tc.tile_pool(name="stg", bufs=2) as spool,
            tc.tile_pool(name="pmm", bufs=2, space="PSUM") as pmm,
            tc.tile_pool(name="pc3", bufs=2, space="PSUM") as pc3,
            tc.tile_pool(name="pc4", bufs=2, space="PSUM") as pc4,
        ):
            tF = cpool.tile([H, _F32W], f32, tag="tF")
            tC = cpool.tile([D, _CW], bf16, tag="tC")
            tO = cpool.tile([1, _OW], bf16, tag="tO")
            # dependency-free warm-ups: bind the lazily-loaded ACT function
            # table and the first-use DVE CAST/TENSOR_SCALAR uop penalties
            # (~1.3us + ~1.4us) to the idle preamble window instead of the
            # first real chunk ops
            wz = cpool.tile([H, 4], f32, tag="wz")
            wzb = cpool.tile([H, 2], bf16, tag="wzb")
            nc.vector.memset(wzb[:, 0:2], 0.0)
            nc.vector.memset(wz[:, 0:2], 0.0)
            nc.scalar.activation(wz[:, 1:2], wz[:, 0:1], act.Abs)
            nc.vector.tensor_copy(wz[:, 3:4], wzb[:, 0:1])
            nc.vector.tensor_scalar(
                out=wzb[:, 1:2], in0=wzb[:, 0:1], scalar1=wz[:, 0:1],
                scalar2=None, op0=alu.is_gt,
            )
            # ALL input loads on the single ACT HWDGE ring in criticality
            # order (tO -> tF -> tC head -> scr tails): ring FIFO makes the
            # completion sems fire in that order both in the scheduler's sim
            # and on hardware, so the c3/c4 matmuls (gated by tO) and is_gt
            # (gated by tF's iota) get scheduled ahead of the c1/c2 matmuls,
            # and the 0.5MB scr tails — tC spans partitions 0-31 only
            # (4 AXI ports, ~115GB/s) — can never port-starve the critical
            # head's completion. The SP ring carries only outputs; gpsimd
            # carries nothing (its ops would lock DVE out of the shared
            # SBUF port pair).
            nc.scalar.dma_start(out=tO[:, :], in_=dO[:, :])
            nc.scalar.dma_start(out=tF[:, :], in_=dF[:, :])
            nc.scalar.dma_start(out=tC[:, 0:1024], in_=dC[:, 0:1024])
            nc.scalar.dma_start(out=tC[:, 1024:2560], in_=dC[:, 1024:2560])

            iota = tF[:, 0:1]
            stamp = tF[:, 1 : 1 + W]
            scatR = tC[:, _SCATR0 : _SCATR0 + W]
            onehotR = tC[:, _ONEHOTR0 : _ONEHOTR0 + W]
            negRow = tC[:, _NEGROW0 : _NEGROW0 + W]
            ones = tO[:, _ONES0 : _ONES0 + 128]

            planes = cpool.tile([H, BPC * FP], f32, tag="planes")

            base = 0
            for ci, gs in enumerate(GROUP_SIZES):
                v = planes[:, base * FP : (base + gs) * FP]
                v4 = v.rearrange("p (g w c) -> p g w c", g=gs, c=C)
                gd = gs * D

                # ---- matmuls: c4 (bh) and c3 (A-B) first — they only need
                # the tiny tO blob and unblock is_gt/Abs early in the ramp —
                # then c1/c2 into t12. c3 and c4 use SEPARATE tiles so the
                # is_gt only waits on the c4 matmul, not the whole group.
                t12 = pmm.tile([H, gs * 2 * W], f32, tag="p12")
                t3 = pc3.tile([H, gd], f32, tag="p3")
                t4 = pc4.tile([H, gd], f32, tag="p4")
                nc.tensor.matmul(
                    t4[:, 0:gd],
                    ones,
                    tO[:, _BH0 + base * D : _BH0 + (base + gs) * D],
                )
                nc.tensor.matmul(
                    t3[:, 0:gd],
                    ones,
                    tO[:, _XS0 + base * D : _XS0 + (base + gs) * D],
                    start=True,
                    stop=False,
                )
                nc.tensor.matmul(
                    t3[:, 0:gd],
                    negRow,
                    tC[:, _XST0 + base : _XST0 + base + gs]
                    .unsqueeze(2)
                    .broadcast_to([D, gs, D]),
                    start=False,
                    stop=True,
                )
                nc.tensor.matmul(
                    t12[:, 0 : gs * W],
                    scatR,
                    tC[:, _SCR0 + base * W : _SCR0 + (base + gs) * W],
                )
                nc.tensor.matmul(
                    t12[:, gs * W : gs * 2 * W],
                    onehotR,
                    tC[:, _XT0 + base : _XT0 + base + gs]
                    .unsqueeze(2)
                    .broadcast_to([D, gs, W]),
                )

                # ---- c4 bars: is_gt(bh, h) staged contiguous then scattered
                stage = spool.tile([H, 128], bf16, tag="bars")
                nc.vector.tensor_scalar(
                    out=stage[:, 0:gd],
                    in0=t4[:, 0:gd],
                    scalar1=iota,
                    scalar2=None,
                    op0=alu.is_gt,
                )
                nc.vector.tensor_copy(
                    v4[:, :, 17:111:3, 4],
                    stage[:, 0:gd].rearrange("p (g d) -> p g d", g=gs),
                )
                # ---- c3: |A - B| from PSUM, 4x replicated stride-5 write
                nc.scalar.activation(
                    v4[:, :, :, 3].rearrange("p g (d r) -> p g d r", r=4),
                    t3[:, 0:gd]
                    .rearrange("p (g d) -> p g d", g=gs)
                    .unsqueeze(3)
                    .broadcast_to([H, gs, D, 4]),
                    act.Abs,
                )
                # ---- c0 stamp on ACT. NEVER put tensor work on gpsimd here:
                # DVE 2-port perf-mode ops (copy/cast/tensor_scalar) and
                # GpSimd ops take an exclusive lock on a shared SBUF port
                # pair, and a gapless burst of gpsimd copies starves DVE for
                # the whole burst (measured 26us on a single CAST).
                nc.scalar.activation(
                    v4[:, :, :, 0],
                    stamp.unsqueeze(1).broadcast_to([H, gs, W]),
                    act.Copy,
                )
                if ci == 1:
                    nc.scalar.dma_start(
                        out=tC[:, 2560:_CW], in_=dC[:, 2560:_CW]
                    )

                # ---- c1+c2: interleaved runs-of-2 copies on DVE (ACT is
                # loaded with abs+stamp; DVE runs at full rate now that
                # gpsimd no longer holds the shared port pair)
                pv = t12[:, :].rearrange("p (c g w) -> p g w c", c=2, w=W)
                nc.vector.tensor_copy(v4[:, :, :, 1:3], pv[:, :])
                # gap zeros: stage*0 instead of memset — the fake data dep on
                # this chunk's is_gt output keeps the scheduler from
                # front-loading all 51 dep-free zero-writes onto DVE ahead of
                # the latency-critical is_gt/bars ops (two edge writes + one
                # merged mid-gap runs-of-2 write)
                z1 = stage[:, 0:1].unsqueeze(1)
                nc.vector.tensor_scalar(
                    out=v4[:, :, 0:17, 4],
                    in0=z1.broadcast_to([H, gs, 17]),
                    scalar1=0.0, scalar2=None, op0=alu.mult,
                )
                nc.vector.tensor_scalar(
                    out=v4[:, :, 111:128, 4],
                    in0=z1.broadcast_to([H, gs, 17]),
                    scalar1=0.0, scalar2=None, op0=alu.mult,
                )
                nc.vector.tensor_scalar(
                    out=v4[:, :, 18:111, 4].rearrange(
                        "p g (k t) -> p g k t", t=3
                    )[:, :, :, 0:2],
                    in0=z1.unsqueeze(3).broadcast_to([H, gs, 31, 2]),
                    scalar1=0.0, scalar2=None, op0=alu.mult,
                )
                # ---- chunk out on the SP ring
                nc.sync.dma_start(
                    out=out_d[base : base + gs, :, :].rearrange("b h f -> h b f"),
                    in_=v.rearrange("p (g f) -> p g f", g=gs),
                )
                base += gs
    nc.finalize()
    return nc


def _host_inputs(inputs, stamp, coords):
    """Build the 8 per-core input maps (f32 + bf16 blobs)."""
    import ml_dtypes

    bf = ml_dtypes.bfloat16
    x = np.ascontiguousarray(inputs, dtype=np.float32)
    stamp2d = np.ascontiguousarray(np.asarray(stamp).reshape(H, W), np.float32)
    coords = np.asarray(coords)

    scatR = np.zeros((D, H), np.float32)
    scatC = np.zeros((D, W), np.float32)
    scatR[np.arange(D), coords[:, 0]] = 1.0
    scatC[np.arange(D), coords[:, 1]] = 1.0
    row_idx = np.repeat(np.arange(D), H // D)
    onehotR = np.zeros((D, H), np.float32)
    onehotR[row_idx, np.arange(H)] = 1.0
    negRow = np.zeros((D, H), np.float32)
    negRow[row_idx, np.arange(H)] = -1.0

    tFb = np.zeros((H, _F32W), np.float32)
    tFb[:, 0] = np.arange(H, dtype=np.float32)
    tFb[:, 1 : 1 + W] = stamp2d

    maps = []
    for m in range(NCORES):
        xm = x[m * BPC : (m + 1) * BPC]                      # [64, 32]
        mn = xm.min(axis=1, keepdims=True)
        mx = xm.max(axis=1, keepdims=True)
        xs = (xm - mn) / (mx - mn)                           # [64, 32] in [0,1]
        bh = np.clip(np.round(xm * np.float32(128.0)), 0, 128)

        tCb = np.zeros((D, _CW), bf)
        tCb[:, _SCATR0 : _SCATR0 + H] = scatR.astype(bf)
        tCb[:, _ONEHOTR0 : _ONEHOTR0 + H] = onehotR.astype(bf)
        tCb[:, _XT0 : _XT0 + BPC] = xm.T.astype(bf)
        tCb[:, _NEGROW0 : _NEGROW0 + H] = negRow.astype(bf)
        tCb[:, _XST0 : _XST0 + BPC] = xs.T.astype(bf)
        scr = (scatC[:, None, :] * xm.T[:, :, None]).reshape(D, BPC * W)
        tCb[:, _SCR0 : _SCR0 + BPC * W] = scr.astype(bf)

        tOb = np.zeros((1, _OW), bf)
        tOb[0, _XS0 : _XS0 + BPC * D] = xs.reshape(-1).astype(bf)
        tOb[0, _BH0 : _BH0 + BPC * D] = bh.reshape(-1).astype(bf)
        tOb[0, _ONES0 : _ONES0 + 128] = np.ones(128, bf)

        maps.append({"df": tFb, "dc": tCb, "do": tOb})
    return maps


class _Runner:
    """Builds the Bass program once and caches the jitted SPMD executable."""

    def __init__(self):
        self.nc = _build_nc()
        self._sharded = None
        self._meta = None

    def _build_exec(self):
        import jax
        import numpy as np
        import concourse.mybir as mybir
        from concourse import bass2jax
        from jax.sharding import Mesh, PartitionSpec
        from jax.experimental.shard_map import shard_map

        bass2jax.install_neuronx_cc_hook()
        nc = self.nc
        partition_name = (
            nc.partition_id_tensor.name if nc.partition_id_tensor else None
        )
        in_names, out_names, out_avals, zero_shapes = [], [], [], []
        for alloc in nc.m.functions[0].allocations:
            if not isinstance(alloc, mybir.MemoryLocationSet):
                continue
            name = alloc.memorylocations[0].name
            if alloc.kind == "ExternalInput":
                if name != partition_name:
                    in_names.append(name)
            elif alloc.kind == "ExternalOutput":
                shape = tuple(alloc.tensor_shape)
                dtype = mybir.dt.np(alloc.dtype)
                out_names.append(name)
                out_avals.append(jax.core.ShapedArray(shape, dtype))
                zero_shapes.append((shape, dtype))
        n_params = len(in_names)
        all_names = in_names + out_names
        if partition_name is not None:
            all_names = all_names + [partition_name]
        donate = tuple(range(n_params, n_params + len(out_names)))

        def _body(*args):
            operands = list(args)
            if partition_name is not None:
                operands.append(bass2jax.partition_id_tensor())
            outs = bass2jax._bass_exec_p.bind(
                *operands,
                out_avals=tuple(out_avals),
                in_names=tuple(all_names),
                out_names=tuple(out_names),
                lowering_input_output_aliases=(),
                sim_require_finite=True,
                sim_require_nnan=True,
                nc=nc,
            )
            return tuple(outs)

        devices = jax.devices()[:NCORES]
        mesh = Mesh(np.asarray(devices), ("core",))
        in_specs = (PartitionSpec("core"),) * (n_params + len(out_names))
        out_specs = (PartitionSpec("core"),) * len(out_names)
        sharded = jax.jit(
            shard_map(
                _body,
                mesh=mesh,
                in_specs=in_specs,
                out_specs=out_specs,
                check_rep=False,
            ),
            donate_argnums=donate,
            keep_unused=True,
        )

        # Output buffers are donated bass_exec operands; build them on
        # device (sharded memset) instead of shipping 168MB of host zeros
        # through axon every call.
        import jax.numpy as jnp
        from jax.sharding import NamedSharding

        shardings = tuple(
            NamedSharding(mesh, PartitionSpec("core")) for _ in zero_shapes
        )

        def _make_zeros():
            return tuple(
                jnp.zeros((NCORES * s[0], *s[1:]), dt) for (s, dt) in zero_shapes
            )

        self._zeros_fn = jax.jit(_make_zeros, out_shardings=shardings)
        self._sharded = sharded
        self._meta = (in_names, out_names, zero_shapes)

    def run(self, in_maps):
        if self._sharded is None:
            self._build_exec()
        in_names, out_names, zero_shapes = self._meta
        concat_in = [
            np.concatenate([np.asarray(m[name]) for m in in_maps], axis=0)
            for name in in_names
        ]
        out_arrs = self._sharded(*concat_in, *self._zeros_fn())
        outs = [np.asarray(a) for a in out_arrs]
        per_core = []
        for c in range(NCORES):
            per_core.append(
                {
                    name: outs[i].reshape(NCORES, *zero_shapes[i][0])[c]
                    for i, name in enumerate(out_names)
                }
            )
        return per_core


def _get_runner():
    global _RUNNER
    if _RUNNER is None:
        _RUNNER = _Runner()
    return _RUNNER


def kernel(inputs, stamp, coords):
    inputs = np.asarray(inputs)
    stamp = np.asarray(stamp)
    coords = np.asarray(coords)
    runner = _get_runner()
    in_maps = _host_inputs(inputs, stamp, coords)
    results = runner.run(in_maps)
    out = np.stack([r["out"] for r in results], axis=0)  # [8, 64, H, W*C]
    out = out.reshape(B, H, W, C).astype(np.float32)
    return out


# revision 31
# speedup vs baseline: 1.0464x; 1.0464x over previous
"""DeepInsight encoding kernel for 8 Trainium2 NeuronCores.

Data-parallel over batch: each core builds 64 interleaved [H, W*5] output
planes in one resident SBUF buffer and streams them to HBM in 17 chunks
(2,2,4x15 planes). The 21MB/core output write at ~370GB/s is the roofline;
everything else is organized to keep the output DMA queue saturated from
~10us until the end:

  - input loads ride separate DMA rings (ACT HWDGE ring for the critical
    head, gpsimd SWDGE ring for the bulk scr tail) so they never queue
    behind output packets on the SP ring,
  - per-batch prep that the baseline loaded as 1MB of replicated bf16
    (bh broadcast, c3 column-minus-row) is instead built on-chip with
    k=1 / k=32 matmuls into PSUM (ones- and -onehot stationaries), cutting
    HBM reads to ~0.6MB,
  - c1 scatter + c2 row-copy are two matmuls per chunk into one PSUM
    tile -> interleaved runs-of-2 strided copies split DVE/ACT,
  - c3 |x_col - x_row| accumulated in PSUM (A - B) -> one ACT Abs with
    4x-replicated stride-5 write,
  - c4 bars: is_gt(PSUM, iota) into a contiguous stage tile, then a
    strided copy; gap zeros via 3 DVE memsets (mid gaps merged into one
    runs-of-2 op),
  - stamp channel copied by gpsimd.
"""

import numpy as np

B, D, H, W, C = 512, 32, 128, 128, 5
NCORES = 8
BPC = B // NCORES            # 64 batches per core
FP = W * C                   # 640 floats per output row
GROUP_SIZES = [2, 2] + [4] * 15
assert sum(GROUP_SIZES) == BPC

# tF [128, 129] f32:   iota 0 | stamp 1:129
_F32W = 129
# tC [32, 8704] bf16: scatR 0:128 | onehotR 128:256 | x_t 256:320 |
#                     negRow 320:448 | xsT 448:512 | scr 512:8704
_SCATR0, _ONEHOTR0, _XT0, _NEGROW0, _XST0, _SCR0, _CW = (
    0, 128, 256, 320, 448, 512, 8704)
# tO [1, 4224] bf16: xs_flat 0:2048 | bh_flat 2048:4096 | ones 4096:4224
_XS0, _BH0, _ONES0, _OW = 0, 2048, 4096, 4224

_RUNNER = None


def _build_nc():
    import concourse.bacc as bacc
    import concourse.mybir as mybir
    from concourse.tile import TileContext

    f32 = mybir.dt.float32
    bf16 = mybir.dt.bfloat16
    alu = mybir.AluOpType
    act = mybir.ActivationFunctionType

    nc = bacc.Bacc()
    dF = nc.dram_tensor("df", [H, _F32W], f32, kind="ExternalInput")
    dC = nc.dram_tensor("dc", [D, _CW], bf16, kind="ExternalInput")
    dO = nc.dram_tensor("do", [1, _OW], bf16, kind="ExternalInput")
    out_d = nc.dram_tensor("out", [BPC, H, FP], f32, kind="ExternalOutput")

    with TileContext(nc) as tc:
        with (
            tc.tile_pool(name="const", bufs=1) as cpool,
            tc.tile_pool(name="stg", bufs=2) as spool,
            tc.tile_pool(name="pmm", bufs=2, space="PSUM") as pmm,
            tc.tile_pool(name="pc3", bufs=2, space="PSUM") as pc3,
            tc.tile_pool(name="pc4", bufs=2, space="PSUM") as pc4,
        ):
            tF = cpool.tile([H, _F32W], f32, tag="tF")
            tC = cpool.tile([D, _CW], bf16, tag="tC")
            tO = cpool.tile([1, _OW], bf16, tag="tO")
            # dependency-free warm-ups: bind the lazily-loaded ACT function
            # table and the first-use DVE CAST/TENSOR_SCALAR uop penalties
            # (~1.3us + ~1.4us) to the idle preamble window instead of the
            # first real chunk ops
            wz = cpool.tile([H, 4], f32, tag="wz")
            wzb = cpool.tile([H, 2], bf16, tag="wzb")
            nc.vector.memset(wzb[:, 0:2], 0.0)
            nc.vector.memset(wz[:, 0:2], 0.0)
            nc.scalar.activation(wz[:, 1:2], wz[:, 0:1], act.Abs)
            nc.vector.tensor_copy(wz[:, 3:4], wzb[:, 0:1])
            nc.vector.tensor_scalar(
                out=wzb[:, 1:2], in0=wzb[:, 0:1], scalar1=wz[:, 0:1],
                scalar2=None, op0=alu.is_gt,
            )
            # ALL input loads on the single ACT HWDGE ring in criticality
            # order (tO -> tF -> tC head -> scr tails): ring FIFO makes the
            # completion sems fire in that order both in the scheduler's sim
            # and on hardware, so the c3/c4 matmuls (gated by tO) and is_gt
            # (gated by tF's iota) get scheduled ahead of the c1/c2 matmuls,
            # and the 0.5MB scr tails — tC spans partitions 0-31 only
            # (4 AXI ports, ~115GB/s) — can never port-starve the critical
            # head's completion. The SP ring carries only outputs; gpsimd
            # carries nothing (its ops would lock DVE out of the shared
            # SBUF port pair).
            nc.scalar.dma_start(out=tO[:, :], in_=dO[:, :])
            nc.scalar.dma_start(out=tC[:, 0:1024], in_=dC[:, 0:1024])
            nc.scalar.dma_start(out=tF[:, :], in_=dF[:, :])
            nc.scalar.dma_start(out=tC[:, 1024:2560], in_=dC[:, 1024:2560])

            iota = tF[:, 0:1]
            stamp = tF[:, 1 : 1 + W]
            scatR = tC[:, _SCATR0 : _SCATR0 + W]
            onehotR = tC[:, _ONEHOTR0 : _ONEHOTR0 + W]
            negRow = tC[:, _NEGROW0 : _NEGROW0 + W]
            ones = tO[:, _ONES0 : _ONES0 + 128]

            planes = cpool.tile([H, BPC * FP], f32, tag="planes")

            base = 0
            for ci, gs in enumerate(GROUP_SIZES):
                v = planes[:, base * FP : (base + gs) * FP]
                v4 = v.rearrange("p (g w c) -> p g w c", g=gs, c=C)
                gd = gs * D

                # ---- matmuls: c4 (bh) and c3 (A-B) first — they only need
                # the tiny tO blob and unblock is_gt/Abs early in the ramp —
                # then c1/c2 into t12. c3 and c4 use SEPARATE tiles so the
                # is_gt only waits on the c4 matmul, not the whole group.
                t12 = pmm.tile([H, gs * 2 * W], f32, tag="p12")
                t3 = pc3.tile([H, gd], f32, tag="p3")
                t4 = pc4.tile([H, gd], f32, tag="p4")
                nc.tensor.matmul(
                    t4[:, 0:gd],
                    ones,
                    tO[:, _BH0 + base * D : _BH0 + (base + gs) * D],
                )
                nc.tensor.matmul(
                    t3[:, 0:gd],
                    ones,
                    tO[:, _XS0 + base * D : _XS0 + (base + gs) * D],
                    start=True,
                    stop=False,
                )
                nc.tensor.matmul(
                    t3[:, 0:gd],
                    negRow,
                    tC[:, _XST0 + base : _XST0 + base + gs]
                    .unsqueeze(2)
                    .broadcast_to([D, gs, D]),
                    start=False,
                    stop=True,
                )
                nc.tensor.matmul(
                    t12[:, 0 : gs * W],
                    scatR,
                    tC[:, _SCR0 + base * W : _SCR0 + (base + gs) * W],
                )
                nc.tensor.matmul(
                    t12[:, gs * W : gs * 2 * W],
                    onehotR,
                    tC[:, _XT0 + base : _XT0 + base + gs]
                    .unsqueeze(2)
                    .broadcast_to([D, gs, W]),
                )

                # ---- c4 bars: is_gt(bh, h) staged contiguous then scattered
                stage = spool.tile([H, 128], bf16, tag="bars")
                nc.vector.tensor_scalar(
                    out=stage[:, 0:gd],
                    in0=t4[:, 0:gd],
                    scalar1=iota,
                    scalar2=None,
                    op0=alu.is_gt,
                )
                nc.vector.tensor_copy(
                    v4[:, :, 17:111:3, 4],
                    stage[:, 0:gd].rearrange("p (g d) -> p g d", g=gs),
                )
                # ---- c3: |A - B| from PSUM, 4x replicated stride-5 write
                nc.scalar.activation(
                    v4[:, :, :, 3].rearrange("p g (d r) -> p g d r", r=4),
                    t3[:, 0:gd]
                    .rearrange("p (g d) -> p g d", g=gs)
                    .unsqueeze(3)
                    .broadcast_to([H, gs, D, 4]),
                    act.Abs,
                )
                # ---- c0 stamp on ACT. NEVER put tensor work on gpsimd here:
                # DVE 2-port perf-mode ops (copy/cast/tensor_scalar) and
                # GpSimd ops take an exclusive lock on a shared SBUF port
                # pair, and a gapless burst of gpsimd copies starves DVE for
                # the whole burst (measured 26us on a single CAST).
                nc.scalar.activation(
                    v4[:, :, :, 0],
                    stamp.unsqueeze(1).broadcast_to([H, gs, W]),
                    act.Copy,
                )
                if ci == 1:
                    nc.scalar.dma_start(
                        out=tC[:, 2560:_CW], in_=dC[:, 2560:_CW]
                    )

                # ---- c1+c2: interleaved runs-of-2 copies on DVE (ACT is
                # loaded with abs+stamp; DVE runs at full rate now that
                # gpsimd no longer holds the shared port pair)
                pv = t12[:, :].rearrange("p (c g w) -> p g w c", c=2, w=W)
                nc.vector.tensor_copy(v4[:, :, :, 1:3], pv[:, :])
                # gap zeros: stage*0 instead of memset — the fake data dep on
                # this chunk's is_gt output keeps the scheduler from
                # front-loading all 51 dep-free zero-writes onto DVE ahead of
                # the latency-critical is_gt/bars ops (two edge writes + one
                # merged mid-gap runs-of-2 write)
                z1 = stage[:, 0:1].unsqueeze(1)
                nc.vector.tensor_scalar(
                    out=v4[:, :, 0:17, 4],
                    in0=z1.broadcast_to([H, gs, 17]),
                    scalar1=0.0, scalar2=None, op0=alu.mult,
                )
                nc.vector.tensor_scalar(
                    out=v4[:, :, 111:128, 4],
                    in0=z1.broadcast_to([H, gs, 17]),
                    scalar1=0.0, scalar2=None, op0=alu.mult,
                )
                nc.vector.tensor_scalar(
                    out=v4[:, :, 18:111, 4].rearrange(
                        "p g (k t) -> p g k t", t=3
                    )[:, :, :, 0:2],
                    in0=z1.unsqueeze(3).broadcast_to([H, gs, 31, 2]),
                    scalar1=0.0, scalar2=None, op0=alu.mult,
                )
                # ---- chunk out on the SP ring
                nc.sync.dma_start(
                    out=out_d[base : base + gs, :, :].rearrange("b h f -> h b f"),
                    in_=v.rearrange("p (g f) -> p g f", g=gs),
                )
                base += gs
    nc.finalize()
    return nc


def _host_inputs(inputs, stamp, coords):
    """Build the 8 per-core input maps (f32 + bf16 blobs)."""
    import ml_dtypes

    bf = ml_dtypes.bfloat16
    x = np.ascontiguousarray(inputs, dtype=np.float32)
    stamp2d = np.ascontiguousarray(np.asarray(stamp).reshape(H, W), np.float32)
    coords = np.asarray(coords)

    scatR = np.zeros((D, H), np.float32)
    scatC = np.zeros((D, W), np.float32)
    scatR[np.arange(D), coords[:, 0]] = 1.0
    scatC[np.arange(D), coords[:, 1]] = 1.0
    row_idx = np.repeat(np.arange(D), H // D)
    onehotR = np.zeros((D, H), np.float32)
    onehotR[row_idx, np.arange(H)] = 1.0
    negRow = np.zeros((D, H), np.float32)
    negRow[row_idx, np.arange(H)] = -1.0

    tFb = np.zeros((H, _F32W), np.float32)
    tFb[:, 0] = np.arange(H, dtype=np.float32)
    tFb[:, 1 : 1 + W] = stamp2d

    maps = []
    for m in range(NCORES):
        xm = x[m * BPC : (m + 1) * BPC]                      # [64, 32]
        mn = xm.min(axis=1, keepdims=True)
        mx = xm.max(axis=1, keepdims=True)
        xs = (xm - mn) / (mx - mn)                           # [64, 32] in [0,1]
        bh = np.clip(np.round(xm * np.float32(128.0)), 0, 128)

        tCb = np.zeros((D, _CW), bf)
        tCb[:, _SCATR0 : _SCATR0 + H] = scatR.astype(bf)
        tCb[:, _ONEHOTR0 : _ONEHOTR0 + H] = onehotR.astype(bf)
        tCb[:, _XT0 : _XT0 + BPC] = xm.T.astype(bf)
        tCb[:, _NEGROW0 : _NEGROW0 + H] = negRow.astype(bf)
        tCb[:, _XST0 : _XST0 + BPC] = xs.T.astype(bf)
        scr = (scatC[:, None, :] * xm.T[:, :, None]).reshape(D, BPC * W)
        tCb[:, _SCR0 : _SCR0 + BPC * W] = scr.astype(bf)

        tOb = np.zeros((1, _OW), bf)
        tOb[0, _XS0 : _XS0 + BPC * D] = xs.reshape(-1).astype(bf)
        tOb[0, _BH0 : _BH0 + BPC * D] = bh.reshape(-1).astype(bf)
        tOb[0, _ONES0 : _ONES0 + 128] = np.ones(128, bf)

        maps.append({"df": tFb, "dc": tCb, "do": tOb})
    return maps


class _Runner:
    """Builds the Bass program once and caches the jitted SPMD executable."""

    def __init__(self):
        self.nc = _build_nc()
        self._sharded = None
        self._meta = None

    def _build_exec(self):
        import jax
        import numpy as np
        import concourse.mybir as mybir
        from concourse import bass2jax
        from jax.sharding import Mesh, PartitionSpec
        from jax.experimental.shard_map import shard_map

        bass2jax.install_neuronx_cc_hook()
        nc = self.nc
        partition_name = (
            nc.partition_id_tensor.name if nc.partition_id_tensor else None
        )
        in_names, out_names, out_avals, zero_shapes = [], [], [], []
        for alloc in nc.m.functions[0].allocations:
            if not isinstance(alloc, mybir.MemoryLocationSet):
                continue
            name = alloc.memorylocations[0].name
            if alloc.kind == "ExternalInput":
                if name != partition_name:
                    in_names.append(name)
            elif alloc.kind == "ExternalOutput":
                shape = tuple(alloc.tensor_shape)
                dtype = mybir.dt.np(alloc.dtype)
                out_names.append(name)
                out_avals.append(jax.core.ShapedArray(shape, dtype))
                zero_shapes.append((shape, dtype))
        n_params = len(in_names)
        all_names = in_names + out_names
        if partition_name is not None:
            all_names = all_names + [partition_name]
        donate = tuple(range(n_params, n_params + len(out_names)))

        def _body(*args):
            operands = list(args)
            if partition_name is not None:
                operands.append(bass2jax.partition_id_tensor())
            outs = bass2jax._bass_exec_p.bind(
                *operands,
                out_avals=tuple(out_avals),
                in_names=tuple(all_names),
                out_names=tuple(out_names),
                lowering_input_output_aliases=(),
                sim_require_finite=True,
                sim_require_nnan=True,
                nc=nc,
            )
            return tuple(outs)

        devices = jax.devices()[:NCORES]
        mesh = Mesh(np.asarray(devices), ("core",))
        in_specs = (PartitionSpec("core"),) * (n_params + len(out_names))
        out_specs = (PartitionSpec("core"),) * len(out_names)
        sharded = jax.jit(
            shard_map(
                _body,
                mesh=mesh,
                in_specs=in_specs,
                out_specs=out_specs,
                check_rep=False,
            ),
            donate_argnums=donate,
            keep_unused=True,
        )

        # Output buffers are donated bass_exec operands; build them on
        # device (sharded memset) instead of shipping 168MB of host zeros
        # through axon every call.
        import jax.numpy as jnp
        from jax.sharding import NamedSharding

        shardings = tuple(
            NamedSharding(mesh, PartitionSpec("core")) for _ in zero_shapes
        )

        def _make_zeros():
            return tuple(
                jnp.zeros((NCORES * s[0], *s[1:]), dt) for (s, dt) in zero_shapes
            )

        self._zeros_fn = jax.jit(_make_zeros, out_shardings=shardings)
        self._sharded = sharded
        self._meta = (in_names, out_names, zero_shapes)

    def run(self, in_maps):
        if self._sharded is None:
            self._build_exec()
        in_names, out_names, zero_shapes = self._meta
        concat_in = [
            np.concatenate([np.asarray(m[name]) for m in in_maps], axis=0)
            for name in in_names
        ]
        out_arrs = self._sharded(*concat_in, *self._zeros_fn())
        outs = [np.asarray(a) for a in out_arrs]
        per_core = []
        for c in range(NCORES):
            per_core.append(
                {
                    name: outs[i].reshape(NCORES, *zero_shapes[i][0])[c]
                    for i, name in enumerate(out_names)
                }
            )
        return per_core


def _get_runner():
    global _RUNNER
    if _RUNNER is None:
        _RUNNER = _Runner()
    return _RUNNER


def kernel(inputs, stamp, coords):
    inputs = np.asarray(inputs)
    stamp = np.asarray(stamp)
    coords = np.asarray(coords)
    runner = _get_runner()
    in_maps = _host_inputs(inputs, stamp, coords)
    results = runner.run(in_maps)
    out = np.stack([r["out"] for r in results], axis=0)  # [8, 64, H, W*C]
    out = out.reshape(B, H, W, C).astype(np.float32)
    return out
